# revision 28
# baseline (speedup 1.0000x reference)
"""Dual (global + local-masked) BERT self-attention on 8 Trainium2 NeuronCores.

Problem: B=2, S=2048, H=1024, NH=16 heads of DH=64.
  q/k/v = hidden @ W{q,k,v}.T + b ; scores = q k^T / 8
  probs_g = softmax(scores + attention_mask)         (additive, zeros in spec)
  probs_l = softmax(scores + (-inf where local_mask==0))
  out     = gate * (probs_l @ v) + (1-gate) * (probs_g @ v)

Sharding: 32 (batch, head) pairs -> 4 heads per core (core c: batch c//4,
heads 4*(c%4)..+4). Each core computes its heads' projections + dual
attention independently; no collectives.

All matmuls bf16 (fp8 was tried: each fp8 tensor in the matmul path costs
~2-5% rel err because the reductions are random-sign — over the 2e-2 gate).

v3 engine plan (per core):
  PE   : bf16 projections, bf16 scores (K=64), bf16 dual-ctx with a ones
         column on V so the softmax denominators ride in psum row 64.
  ACT  : the 128 exp instructions (scores psum f32 -> bf16 e tiles) +
         projection psum copies (startup only, ACT is idle then).
  DVE  : local-branch mask multiply (bf16 all-SBUF -> 2x mode), ctx psum ->
         sbuf copies (frees PSUM early), reciprocal + gate coefficients,
         final combine.
  GP   : gate/denominator coefficient partition_broadcast (SBUF-only; GP
         cannot touch PSUM on real HW).
  DMA  : inputs split across SP/ACT/GP queues (one serial queue stalls the
         per-chunk gt/sums/out DMAs); gt packed per (head, chunk) so the
         whole epilogue coefficient block is one [1, 2*QC] DMA.
  Epilogue runs entirely off-PSUM and overlaps the next chunk's k-loop
  (the old selector-matmul broadcast held the scores psum pool hostage,
  serializing ~12us per chunk).
  No max-subtraction in softmax: scores are O(+-2), exp is safe in f32 and
  softmax is shift-invariant.
Output per core: [256 dims, 2048 q] f32; host transposes/reassembles.
"""

import sys

sys.path.insert(0, "/opt/trn_rl_repo")

import numpy as np
import ml_dtypes

B, S, H, NH, DH = 2, 2048, 1024, 16, 64
NCORES = 8
HPC = 4          # heads per core
MPC = HPC // 2   # head pairs per core
QC = 1024        # query chunk (free dim of scores/ctx psums)
NQC = S // QC
KT = S // 128    # key tiles
XT_T = H // 128  # X^T k-tiles for projections

_BUILT = {}


def _build(use_em: bool, repeat: int = 1, has_b: bool = False):
    from contextlib import ExitStack

    import concourse.mybir as mybir
    from concourse import bacc, tile

    f32 = mybir.dt.float32
    bf16 = mybir.dt.bfloat16
    AF = mybir.ActivationFunctionType

    nc = bacc.Bacc("TRN2", target_bir_lowering=False, debug=False)

    xt_d = nc.dram_tensor("xt", [H, S], bf16, kind="ExternalInput").ap()
    # weights pre-tiled on host to [128, XT_T*256] so each DMA row is 4KB
    wq_d = nc.dram_tensor("wq", [128, XT_T * 256], bf16, kind="ExternalInput").ap()
    wk_d = nc.dram_tensor("wk", [128, XT_T * 256], bf16, kind="ExternalInput").ap()
    wv_d = nc.dram_tensor("wv", [128, XT_T * 256], bf16, kind="ExternalInput").ap()
    bqk_d = nc.dram_tensor("bqk", [2, 256], f32, kind="ExternalInput").ap()
    bv_d = nc.dram_tensor("bv", [1, 256], bf16, kind="ExternalInput").ap()
    # mask as bf16 1.0/0.0, host-transposed to [key, q]
    msk_d = nc.dram_tensor("msk", [KT, 128, S], bf16, kind="ExternalInput").ap()
    # gt[h, qc, r, q]: head h, query chunk qc, r = (gate_h, 1-gate_h)
    gt_d = nc.dram_tensor("gt", [HPC, NQC, 2, QC], f32, kind="ExternalInput").ap()
    if use_em:
        em_d = nc.dram_tensor("em", [KT, 128], f32, kind="ExternalInput").ap()
    out_d = nc.dram_tensor("out", [HPC * DH, S], bf16, kind="ExternalOutput").ap()

    with tile.TileContext(nc) as tc, ExitStack() as ctx:
        big = ctx.enter_context(tc.tile_pool(name="big", bufs=1))

        msk_sb = big.tile([128, KT, S], bf16, name="msk_sb")
        for t in range(KT):
            nc.gpsimd.dma_start(msk_sb[:, t, :], msk_d[t])
        bqk_sb = big.tile([128, 2, 2], f32, name="bqk_sb")
        nc.sync.dma_start(
            bqk_sb, bqk_d.rearrange("c (t p) -> p c t", p=128)
        )
        bv_sb = big.tile([1, 256], bf16, name="bv_sb")
        nc.sync.dma_start(bv_sb, bv_d)
        if use_em:
            em_sb = big.tile([128, KT], f32, name="em_sb")
            nc.sync.dma_start(em_sb, em_d.rearrange("t p -> p t"))

        ones_r = big.tile([1, 128], bf16, name="ones_r")
        nc.vector.memset(ones_r, 1.0)

        qt_sb = big.tile([128, MPC, S], bf16, name="qt_sb")
        kt_sb = big.tile([128, MPC, S], bf16, name="kt_sb")
        # V with a ones column so psum row 64 accumulates the denominator
        v_sb = big.tile([128, KT, HPC, 65], bf16, name="v_sb")
        nc.vector.memset(v_sb[:, :, :, 64:65], 1.0)

        for _rep in range(repeat):
            # ---- projections: Q^T, K^T (transposed), V (natural) ----
            # xt/w live in a transient pool: their 44KB/partition is released
            # before the attention pools open (SBUF cannot hold them plus the
            # resident mask at once).
            proj_ctx = ExitStack()
            pin = proj_ctx.enter_context(tc.tile_pool(name="pin", bufs=1))
            xt_sb = pin.tile([128, XT_T, S], bf16, name="xt_sb")
            for t in range(XT_T):
                nc.scalar.dma_start(xt_sb[:, t, :], xt_d[t * 128:(t + 1) * 128, :])
            w_sbs = {}
            for nm, d in (("wq", wq_d), ("wk", wk_d), ("wv", wv_d)):
                w_sb = pin.tile([128, XT_T, 256], bf16, name=f"{nm}_sb")
                nc.sync.dma_start(w_sb, d.rearrange("p (t s) -> p t s", t=XT_T))
                w_sbs[nm] = w_sb
            def _qk_proj(pproj, m):
                for ci, (wn, dst) in enumerate((("wq", qt_sb), ("wk", kt_sb))):
                    w_sb = w_sbs[wn]
                    for nq in range(S // 1024):
                        ps = pproj.tile([128, 1024], f32, tag="pp")
                        for t in range(XT_T):
                            for hlf in range(2):
                                nc.tensor.matmul(
                                    ps[:, hlf * 512:(hlf + 1) * 512],
                                    lhsT=w_sb[:, t, m * 128:(m + 1) * 128],
                                    rhs=xt_sb[:, t, nq * 1024 + hlf * 512:
                                              nq * 1024 + (hlf + 1) * 512],
                                    start=(t == 0),
                                    stop=(t == XT_T - 1),
                                )
                        nc.scalar.activation(
                            dst[:, m, nq * 1024:(nq + 1) * 1024], ps,
                            AF.Identity, bias=bqk_sb[:, ci, m:m + 1], scale=1.0,
                        )

            # order: head-pair 0 q/k, then V, then pair 1 q/k — so heads 0/1
            # attention can start while pair 1 is still projecting
            with tc.tile_pool(name="pproj", bufs=2, space="PSUM") as pproj:
                _qk_proj(pproj, 0)
            with tc.tile_pool(name="pv", bufs=2, space="PSUM") as pv:
                for st in range(KT):
                    ps = pv.tile([128, 256], f32, tag="pv")
                    for t in range(XT_T):
                        nc.tensor.matmul(
                            ps,
                            lhsT=xt_sb[:, t, st * 128:(st + 1) * 128],
                            rhs=w_sbs["wv"][:, t, :],
                            start=(t == 0),
                            stop=(t == XT_T - 1 and not has_b),
                        )
                    if has_b:
                        nc.tensor.matmul(
                            ps, lhsT=ones_r, rhs=bv_sb, start=False, stop=True
                        )
                    nc.scalar.activation(
                        v_sb[:, st, :, 0:64],
                        ps.rearrange("p (h d) -> p h d", h=HPC),
                        AF.Copy,
                    )
            with tc.tile_pool(name="pproj2", bufs=2, space="PSUM") as pproj2:
                _qk_proj(pproj2, 1)
            proj_ctx.close()

            # ---- dual attention ----
            att_ctx = ExitStack()
            psc = att_ctx.enter_context(tc.tile_pool(name="psc", bufs=2, space="PSUM"))
            pctx = att_ctx.enter_context(tc.tile_pool(name="pctx", bufs=1, space="PSUM"))
            pe = att_ctx.enter_context(tc.tile_pool(name="pe", bufs=4))
            pt = att_ctx.enter_context(tc.tile_pool(name="pt", bufs=2))
            po = att_ctx.enter_context(tc.tile_pool(name="po", bufs=2))
            pc = att_ctx.enter_context(tc.tile_pool(name="pc", bufs=8))

            for h in range(HPC):
                m, par = h // 2, h % 2
                ksl = slice(64 * par, 64 * par + 64)  # head's dims within pair
                for qc in range(NQC):
                    qs = slice(qc * QC, (qc + 1) * QC)
                    gtt = pc.tile([1, 2, QC], f32, name="gtt", tag="gtt", bufs=2)
                    nc.gpsimd.dma_start(gtt, gt_d[h:h + 1, qc])
                    ctg = pctx.tile([65, QC], f32, name="ctg", tag="ctxg")
                    ctl = pctx.tile([65, QC], f32, name="ctl", tag="ctxl")

                    def _ctx_mm(eg, el, t):
                        st0 = (t == 0)
                        st1 = (t == KT - 1)
                        for hlf in range(QC // 512):
                            h5 = slice(hlf * 512, (hlf + 1) * 512)
                            nc.tensor.matmul(ctg[:, h5], lhsT=v_sb[:, t, h, :],
                                             rhs=eg[:, h5], start=st0, stop=st1)
                            nc.tensor.matmul(ctl[:, h5], lhsT=v_sb[:, t, h, :],
                                             rhs=el[:, h5], start=st0, stop=st1)

                    # software-pipelined: scores(t) is emitted before ctx(t-1)
                    # so a stalled ctx never blocks the next exp in the PE queue
                    prev = None
                    for t in range(KT):
                        ps = psc.tile([128, QC], f32, name="ps", tag="sc")
                        for hlf in range(QC // 512):
                            nc.tensor.matmul(
                                ps[:, hlf * 512:(hlf + 1) * 512],
                                lhsT=kt_sb[ksl, m, t * 128:(t + 1) * 128],
                                rhs=qt_sb[ksl, m, qc * QC + hlf * 512:
                                          qc * QC + (hlf + 1) * 512],
                                start=True, stop=True,
                            )
                        e = pe.tile([128, QC], bf16, name="e", tag="e")
                        nc.scalar.activation(e, ps, AF.Exp)
                        el = pe.tile([128, QC], bf16, name="el", tag="el")
                        nc.vector.tensor_mul(el, e, msk_sb[:, t, qs])
                        if use_em:
                            eg = pe.tile([128, QC], bf16, name="eg", tag="e")
                            nc.vector.tensor_scalar_mul(eg, e, em_sb[:, t:t + 1])
                        else:
                            eg = e
                        if prev is not None:
                            _ctx_mm(*prev)
                        prev = (eg, el, t)
                    _ctx_mm(*prev)
                    # epilogue: free ctx PSUM early via DVE f32 copies, then
                    # normalize+gate from SBUF while the next chunk accumulates.
                    ctl_s = pt.tile([65, QC], f32, name="ctl_s", tag="cts")
                    ctg_s = pt.tile([65, QC], f32, name="ctg_s", tag="cts")
                    nc.vector.tensor_copy(ctl_s, ctl)
                    nc.vector.tensor_copy(ctg_s, ctg)
                    sums2 = pc.tile([1, 2, QC], f32, name="sums2", tag="sums", bufs=2)
                    nc.sync.dma_start(sums2[0:1, 0, :], ctl_s[64:65, :])
                    nc.sync.dma_start(sums2[0:1, 1, :], ctg_s[64:65, :])
                    rec2 = pc.tile([1, 2, QC], f32, name="rec2", tag="sums", bufs=2)
                    nc.vector.reciprocal_approx_fast(rec2, sums2)
                    coef2 = pc.tile([1, 2, QC], bf16, name="coef2", tag="coefb", bufs=2)
                    nc.vector.tensor_mul(coef2, rec2, gtt)
                    bc_b = pt.tile([64, 2, QC], bf16, name="bc_b", tag="bc")
                    nc.gpsimd.partition_broadcast(bc_b, coef2)
                    t1 = pt.tile([64, QC], f32, name="t1", tag="t")
                    t2 = pt.tile([64, QC], f32, name="t2", tag="t")
                    nc.vector.tensor_mul(t1, ctl_s[0:64, :], bc_b[:, 0, :])
                    nc.vector.tensor_mul(t2, ctg_s[0:64, :], bc_b[:, 1, :])
                    o = po.tile([64, QC], bf16, name="o", tag="o")
                    nc.vector.tensor_add(o, t1, t2)
                    nc.gpsimd.dma_start(out_d[h * 64:(h + 1) * 64, qs], o)
            att_ctx.close()

    nc.compile()
    return nc


def _get(use_em: bool, has_b: bool):
    key = (use_em, has_b)
    if key not in _BUILT:
        _BUILT[key] = _build(use_em, has_b=has_b)
    return _BUILT[key]


def _prep_core(c, hs, am, lm, go, Wq, bq, Wk, bk, Wv, bv, use_em):
    bf = ml_dtypes.bfloat16
    b, hg = c // 4, c % 4
    h0 = hg * HPC
    sl = slice(h0 * DH, (h0 + HPC) * DH)

    def _wtile(w):  # scaled [256, H] slice -> [128, XT_T*256] partition-tiled
        return np.ascontiguousarray(
            w.T.reshape(XT_T, 128, 256).transpose(1, 0, 2).reshape(128, -1))

    m = {
        "xt": np.ascontiguousarray(hs[b].T).astype(bf),
        "wq": _wtile(Wq[sl, :] / 8.0).astype(bf),
        "wk": _wtile(Wk[sl, :]).astype(bf),
        "wv": _wtile(Wv[sl, :]).astype(bf),
        "bqk": np.stack([bq[sl] / 8.0, bk[sl]]).astype(np.float32),
        "bv": bv[sl].reshape(1, 256).astype(bf),
        "msk": np.ascontiguousarray(
            lm[b, 0].astype(np.float32).T).reshape(KT, 128, S).astype(bf),
        "gt": np.stack([
            np.stack([go[b, h0 + j, :, 0], 1.0 - go[b, h0 + j, :, 0]])
            for j in range(HPC)
        ]).reshape(HPC, 2, NQC, QC).transpose(0, 2, 1, 3).copy().astype(np.float32),
    }
    if use_em:
        m["em"] = np.exp(am[b, 0, 0]).astype(np.float32).reshape(KT, 128)
    return m


def make_in_maps(inputs):
    hs = np.asarray(inputs["hidden_states"], np.float32)
    am = np.asarray(inputs["attention_mask"], np.float32)
    lm = np.asarray(inputs["local_attention_mask"])
    go = np.asarray(inputs["gate_outputs"], np.float32)
    Wq = np.asarray(inputs["Wq"], np.float32)
    bq = np.asarray(inputs["bq"], np.float32)
    Wk = np.asarray(inputs["Wk"], np.float32)
    bk = np.asarray(inputs["bk"], np.float32)
    Wv = np.asarray(inputs["Wv"], np.float32)
    bv = np.asarray(inputs["bv"], np.float32)
    use_em = bool(np.any(am != 0.0))
    has_b = bool(np.any(bq != 0.0) or np.any(bk != 0.0) or np.any(bv != 0.0))
    maps = [
        _prep_core(c, hs, am, lm, go, Wq, bq, Wk, bk, Wv, bv, use_em)
        for c in range(NCORES)
    ]
    return maps, (use_em, has_b)


def assemble(results):
    out = np.empty((B, S, H), np.float32)
    for c in range(NCORES):
        b, hg = c // 4, c % 4
        sl = slice(hg * HPC * DH, (hg + 1) * HPC * DH)
        out[b, :, sl] = np.asarray(results[c]["out"]).T
    return out


def kernel(**inputs):
    from concourse import bass_utils

    maps, (use_em, has_b) = make_in_maps(inputs)
    nc = _get(use_em, has_b)
    res = bass_utils.run_bass_kernel_spmd(nc, maps, core_ids=list(range(NCORES)))
    return assemble(res.results)
